# revision 39
# baseline (speedup 1.0000x reference)
"""Trainium2 Bass kernel for nn_AttentionBlock (GroupNorm + windowed MHA + proj + residual).

Contract: kernel(**inputs) takes FULL unsharded inputs (as from reference.setup_inputs())
and returns the FULL output [1, 256, 96, 96] float32.

Sharding: sequence-parallel over query positions across 8 cores. Each core gets a
uniform slice of each of the 3 reference attention windows:
  W0: q[512i   : 512(i+1)]    attends kv[0    : 6144]
  W1: q[4096+512i : ...]      attends kv[2048 : 9216]
  W2: q[8192+128i : ...]      attends kv[6144 : 9216]
All 4 heads for those queries are computed locally, so the output projection and
residual are local too. Every core redundantly computes GroupNorm stats and the
full-sequence K/V (its key windows span the whole sequence).

Phase structure (single core):
  1. stats: x [256,9216] f32 streamed in on 4 DMA queues; bn_stats on DVE;
     ScalarE copies each tile to a resident fp16 buffer (xn).
  2. GroupNorm reduce -> per-channel a,b (tiny PE matmuls against group maps).
  3. xn normalized IN PLACE on DVE (fp16, 2x/4x DVE modes); xq -> xnq fp16.
  4. qkv for ALL 4 heads once: K^T [2 head-pairs x 128, seq] fp16, V [token-chunk,
     head, 64|1] fp16 (65th col ones -> softmax sums fall out of the PV matmul),
     q per window fp16. All matmul operands fp16 (PE streams 2-byte lhsT faster;
     fp32 PSUM accumulation keeps precision; rel err ~1e-3 << 2e-2 gate).
  5. Attention sweep per head-pair hp: for each window, stream key-chunks in
     SCHUNK=2 groups: QK pair (2 heads interleaved on PE row-tiles 0/64 -> they
     run concurrently), exp on ScalarE (PSUM->SBUF fp16), PV pair accumulating
     O^T[65, qn] per head in PSUM. QK is emitted ONE GROUP AHEAD of exp/PV:
     the PE executes in order, so group g's PV (gated on exp g) must sit after
     group g+1's QK or the PE idles every group and HAM-downclocks.
     Epilogue per window: sums row -> PE ones-broadcast -> DVE reciprocal+mult
     -> attn [64, head, q] fp16 (everything stays on partitions 0:64 so no
     partition-moving DMAs). Epilogues/projections are injected into the next
     window's group loop so they overlap attention instead of serializing.
  6. Projection per window (after sweep 1): K=64 matmuls per head against a
     head-major projTh layout + rank-1 bias matmul + residual, DMA out.
"""

import numpy as np

import concourse.bass as bass
import concourse.tile as tile
from concourse import mybir
from concourse.vector_clock import ScopedClock, VectorClock

F32 = mybir.dt.float32
F32R = mybir.dt.float32r
F16 = mybir.dt.float16
AF = mybir.ActivationFunctionType
ALU = mybir.AluOpType

C = 256
SEQ = 9216
NCORES = 8
HEADS = 4
D = 64
EPS = 1e-5
SCALE = 0.125  # 1/sqrt(64)
NQC = 1152  # queries per core
ST = 512  # seq tile for qkv
NST = SEQ // ST  # 18
NCH = SEQ // 128  # 72 key chunks
# windows: (q_off, q_len, key_chunk0, n_key_chunks)
WINDOWS = [(0, 512, 0, 48), (512, 512, 16, 56), (1024, 128, 48, 24)]
SCHUNK = 2  # key-chunk items batched per exp ACTIVATE (2 PSUM banks)
ABLATE = None  # timing experiments only: "noqkv" | "noattn" | None (full)


def _patch_tile_drain():
    """This container's walrus rejects >1 sem wait on one sync CTRL instruction
    ("Too many sync wait commands"). Split the TileContext-exit drain's waits
    into one-wait-per-nop instructions."""
    if getattr(tile.TileContext, "_drain_split_patched", False):
        return

    def _drain_and_barrier(self, tick_clock, wait_clock):
        vc = tick_clock.global_clock
        n = len(vc)
        for p in range(n):
            t = vc[p]
            if t <= 0:
                continue
            single = VectorClock([t if i == p else 0 for i in range(n)])
            inst = self.nc.sync.nop(nofuse=True, hint="drain_split_wait")
            wait_clock.add_sem_waits(inst.ins, ScopedClock({None: single}))
        self.nc.sync.drain()
        self.nc.all_engine_barrier()
        assert self.sems is not None
        popped = self.nc._tile_sem_poison_stack.pop()
        assert popped is self._sem_poison
        self.nc.clear_and_free_semaphores(list(self.sems.allocated().values()))
        self.nc.all_engine_barrier()

    tile.TileContext._drain_and_barrier = _drain_and_barrier
    tile.TileContext._drain_split_patched = True


def _patch_to_json_split_waits():
    """This walrus build rejects instructions carrying more than one sem-wait
    ("Too many sync wait commands"). Post-process the BIR JSON: keep one wait
    on each instruction and move extras onto same-engine NoOps inserted just
    before it (identical sync semantics: the engine blocks on the nops first)."""
    if getattr(bass.Bass, "_split_waits_patched", False):
        return
    import json as _json

    orig = bass.Bass.to_json_bytes

    def to_json_bytes(self):
        d = _json.loads(orig(self))
        for fn in d["functions"]:
            for blk in fn["blocks"]:
                out = []
                changed = False
                for ins in blk["instructions"]:
                    si = ins.get("sync_info")
                    waits = (si or {}).get("on_wait") or []
                    if len(waits) > 1:
                        changed = True
                        for k, w in enumerate(waits[:-1]):
                            out.append({
                                "debug": ins.get("debug", 0),
                                "engine": ins["engine"],
                                "ins": [],
                                "name": f"{ins['name']}-w{k}",
                                "opcode": "NoOp",
                                "outs": [],
                                "sync_info": {"on_update": [], "on_wait": [w]},
                                "text_hint": "split_wait",
                            })
                        si["on_wait"] = [waits[-1]]
                    out.append(ins)
                if changed:
                    blk["instructions"] = out
        return _json.dumps(d).encode()

    bass.Bass.to_json_bytes = to_json_bytes
    bass.Bass._split_waits_patched = True


def declare_drams(nc):
    d = {}
    d["x"] = nc.dram_tensor("x", [C, SEQ], F16, kind="ExternalInput")
    d["xq"] = nc.dram_tensor("xq", [C, NQC], F32, kind="ExternalInput")
    d["wT"] = nc.dram_tensor("wT", [C, 3 * C], F32, kind="ExternalInput")
    d["projTh"] = nc.dram_tensor("projTh", [64, 4 * C], F32, kind="ExternalInput")
    d["pvec"] = nc.dram_tensor("pvec", [128, 8], F32, kind="ExternalInput")
    d["projbr"] = nc.dram_tensor("projbr", [1, C], F32, kind="ExternalInput")
    d["G"] = nc.dram_tensor("G", [128, 16], F32, kind="ExternalInput")
    d["GT"] = nc.dram_tensor("GT", [16, 128], F32, kind="ExternalInput")
    d["out"] = nc.dram_tensor("out", [C, NQC], F32, kind="ExternalOutput")
    return d


def build_nc(reps=1):
    """reps>1 re-emits the whole kernel body back-to-back inside one NEFF —
    used only for timing (amortizes the ~2.5ms axon dispatch cost per
    execution; device time per iteration = slope between two reps values)."""
    nc = bass.Bass()
    d = declare_drams(nc)
    with tile.TileContext(nc) as tc:
        for _rep in range(reps):
            _build_body(nc, tc, d)
    return nc


def _build_body(nc, tc, d):
    x_d, xq_d, wT_d, projTh_d = d["x"], d["xq"], d["wT"], d["projTh"]
    pvec_d, projbr_d, G_d, GT_d, out_d = d["pvec"], d["projbr"], d["G"], d["GT"], d["out"]

    with (
        tc.tile_pool(name="singles", bufs=1) as singles,
        tc.tile_pool(name="pt", bufs=4) as ptp,
        tc.tile_pool(name="epi", bufs=3) as epi,
        tc.tile_pool(name="outp", bufs=2) as outp,
        tc.tile_pool(name="pg", bufs=4) as pg,
    ):
        # ---- constants: DMA + fp16 conversion (staging pool freed after) ----
        pvec_sb = singles.tile([128, 8], F32, tag="pvec")
        nc.sync.dma_start(out=pvec_sb, in_=pvec_d[:, :])
        G_sb = singles.tile([128, 16], F32, tag="G")
        nc.sync.dma_start(out=G_sb, in_=G_d[:, :])
        GT_sb = singles.tile([16, 128], F32, tag="GT")
        nc.sync.dma_start(out=GT_sb, in_=GT_d[:, :])
        xq_sb = singles.tile([128, 2, NQC], F32, tag="xq")
        nc.sync.dma_start(out=xq_sb[:, 0, :], in_=xq_d[0:128, :])
        nc.sync.dma_start(out=xq_sb[:, 1, :], in_=xq_d[128:256, :])

        wT_h = singles.tile([128, 2, 3 * C], F16, tag="wT_h")
        projTh_h = singles.tile([64, 4, C], F16, tag="projTh_h")
        projbr_h = singles.tile([1, C], F16, tag="projbr_h")
        with tc.tile_pool(name="wstage", bufs=1) as wstage:
            # fp16 conversions on ScalarE — DVE is the serial startup chain
            # (bn_stats), ScalarE is idle here
            ws = wstage.tile([128, 2, 3 * C], F32, tag="ws")
            nc.sync.dma_start(out=ws[:, 0, :], in_=wT_d[0:128, :])
            nc.sync.dma_start(out=ws[:, 1, :], in_=wT_d[128:256, :])
            nc.scalar.activation(out=wT_h[:, 0, :], in_=ws[:, 0, :], func=AF.Copy)
            nc.scalar.activation(out=wT_h[:, 1, :], in_=ws[:, 1, :], func=AF.Copy)
            ps = wstage.tile([64, 4 * C], F32, tag="ps")
            nc.sync.dma_start(out=ps, in_=projTh_d[:, :])
            nc.scalar.activation(out=projTh_h.rearrange("p h c -> p (h c)"), in_=ps, func=AF.Copy)
            pbs = wstage.tile([1, C], F32, tag="pbs")
            nc.sync.dma_start(out=pbs, in_=projbr_d[:, :])
            nc.scalar.activation(out=projbr_h, in_=pbs, func=AF.Copy)

        ones_h = singles.tile([1, 512], F16, tag="ones_h")
        nc.vector.memset(ones_h, 1.0)
        ones_c = singles.tile([128, 1], F16, tag="ones_c")
        nc.vector.memset(ones_c, 1.0)

        # ---- persistent fp16 state ----
        xn = singles.tile([128, 2, SEQ], F16, tag="xn")          # raw x, then normalized
        k_res = singles.tile([128, 2, SEQ], F16, tag="k_res")    # [2-head rows, hp, token]
        v_res = singles.tile([128, NCH, 2, 130], F16, tag="v_res")  # [tok%128, chunk, hp, 2x(64|1)]
        q_res = singles.tile([128, 2, 3, 512], F16, tag="q_res")  # [2-head rows, hp, window, q]
        attn_sb = singles.tile([64, HEADS, NQC], F16, tag="attn")  # [dim, head, q]
        xnq = singles.tile([128, 2, NQC], F16, tag="xnq")

        # ones columns of v (col 64 of each head slot) — written once
        v5 = v_res.rearrange("p ch hp (hl c) -> p ch hp hl c", hl=2)
        ones_bc = bass.AP(tensor=ones_c.tensor, offset=ones_c.offset,
                          ap=[list(ones_c.ap[0]), [0, NCH], [0, 2], [0, 2], [1, 1]])
        nc.vector.tensor_copy(out=v5[:, :, :, :, 64:65], in_=ones_bc)

        # ---- phase 1: x (fp16, host-cast) streamed straight into xn on 3
        # DMA queues + bn_stats ----
        stats = singles.tile([128, 2, NST, 6], F32, tag="stats")
        # x on the sync+gpsimd queues only: those also carry the out stores,
        # so in reps-timing builds consecutive iterations queue-serialize
        # (slope ~ single-shot latency), and ScalarE never issues DMAs.
        qeng = [nc.gpsimd, nc.sync]
        for bt in range(NST // 2):  # 9 big tiles of [128, 1024] per cc
            for cc in range(2):
                sl = slice(2 * ST * bt, 2 * ST * (bt + 1))
                qeng[(2 * bt + cc) % 2].dma_start(
                    out=xn[:, cc, sl], in_=x_d[128 * cc:128 * (cc + 1), sl])
                nc.vector.bn_stats(out=stats[:, cc, 2 * bt, :],
                                   in_=xn[:, cc, 2 * ST * bt:2 * ST * bt + ST])
                nc.vector.bn_stats(out=stats[:, cc, 2 * bt + 1, :],
                                   in_=xn[:, cc, 2 * ST * bt + ST:2 * ST * (bt + 1)])

        # ---- phase 2: GroupNorm stats -> per-channel a, b ----
        ab_sb = singles.tile([128, 2, 2], F32, tag="ab")  # [:, cc, 0]=a, [:, cc, 1]=b
        gn_scope = tc.tile_pool(name="gnps", bufs=2, space="PSUM")
        gnps = gn_scope.__enter__()
        for cc in range(2):
            mv = pg.tile([128, 2], F32, tag="mv")
            nc.vector.bn_aggr(out=mv, in_=stats[:, cc, :, :])
            st2 = pg.tile([128, 2], F32, tag="st2")  # (mean, E[x^2])
            nc.vector.tensor_copy(out=st2[:, 0:1], in_=mv[:, 0:1])
            nc.vector.tensor_tensor(out=st2[:, 1:2], in0=mv[:, 0:1], in1=mv[:, 0:1], op=ALU.mult)
            nc.vector.tensor_tensor(out=st2[:, 1:2], in0=st2[:, 1:2], in1=mv[:, 1:2], op=ALU.add)
            gps = gnps.tile([128, 512], F32, tag="acc")
            nc.tensor.matmul(gps[0:16, 0:2], lhsT=G_sb, rhs=st2, start=True, stop=True)
            gm = pg.tile([16, 2], F32, tag="gm")  # (mean_g, E2_g)
            nc.vector.tensor_copy(out=gm, in_=gps[0:16, 0:2])
            t16 = pg.tile([16, 1], F32, tag="t16")
            nc.vector.tensor_tensor(out=t16, in0=gm[:, 0:1], in1=gm[:, 0:1], op=ALU.mult)
            nc.vector.tensor_tensor(out=gm[:, 1:2], in0=gm[:, 1:2], in1=t16, op=ALU.subtract)
            # rstd = 1/sqrt(var+eps)
            nc.vector.tensor_scalar_add(out=gm[:, 1:2], in0=gm[:, 1:2], scalar1=EPS)
            nc.scalar.activation(out=gm[:, 1:2], in_=gm[:, 1:2], func=AF.Sqrt)
            nc.vector.reciprocal(out=gm[:, 1:2], in_=gm[:, 1:2])
            mps = gnps.tile([128, 512], F32, tag="acc")
            nc.tensor.matmul(mps[0:128, 0:2], lhsT=GT_sb, rhs=gm, start=True, stop=True)
            mr = pg.tile([128, 2], F32, tag="mr")  # (mean_c, rstd_c)
            nc.vector.tensor_copy(out=mr, in_=mps[0:128, 0:2])
            # a = rstd * norm_w ; b = norm_b - mean * a
            nc.vector.tensor_tensor(out=ab_sb[:, cc, 0:1], in0=mr[:, 1:2], in1=pvec_sb[:, 4 + cc:5 + cc], op=ALU.mult)
            t128 = pg.tile([128, 1], F32, tag="t128")
            nc.vector.tensor_tensor(out=t128, in0=mr[:, 0:1], in1=ab_sb[:, cc, 0:1], op=ALU.mult)
            nc.vector.tensor_tensor(out=ab_sb[:, cc, 1:2], in0=pvec_sb[:, 6 + cc:7 + cc], in1=t128, op=ALU.subtract)
        gn_scope.__exit__(None, None, None)

        # ---- phase 3: xq -> xnq (xn normalized per-tile inside the qkv loop) ----
        for cc in range(2):
            nc.vector.tensor_scalar(
                out=xnq[:, cc, :], in0=xq_sb[:, cc, :],
                scalar1=ab_sb[:, cc, 0:1], scalar2=ab_sb[:, cc, 1:2],
                op0=ALU.mult, op1=ALU.add)

        # ---- phase 4: qkv for all 4 heads ----
        def emit_qkv_k(st, pool):
            s0 = ST * st
            for cc in range(2):  # normalize this tile's xn slice in place (fp16 2x DVE)
                nc.vector.tensor_scalar(
                    out=xn[:, cc, s0:s0 + ST], in0=xn[:, cc, s0:s0 + ST],
                    scalar1=ab_sb[:, cc, 0:1], scalar2=ab_sb[:, cc, 1:2],
                    op0=ALU.mult, op1=ALU.add)
            for kb in range(2):  # k rows [128kb:128kb+128] = head-pair kb
                kps = pool.tile([128, 512], F32, tag="acc", name="kps")
                for cc in range(2):
                    nc.tensor.matmul(
                        kps, lhsT=wT_h[:, cc, C + 128 * kb:C + 128 * kb + 128],
                        rhs=xn[:, cc, s0:s0 + ST], start=(cc == 0), stop=(cc == 1))
                nc.vector.tensor_scalar_add(
                    out=k_res[:, kb, s0:s0 + ST], in0=kps,
                    scalar1=pvec_sb[:, 2 + kb:3 + kb])

        def emit_qkv_v(st, pool):
            s0 = ST * st
            for mc in range(4):  # token sub-chunks of 128
                vps = pool.tile([128, 512], F32, tag="acc", name="vps")
                for cc in range(2):
                    nc.tensor.matmul(
                        vps[:, 0:256],
                        lhsT=xn[:, cc, s0 + 128 * mc:s0 + 128 * (mc + 1)],
                        rhs=wT_h[:, cc, 2 * C:3 * C],
                        start=(cc == 0), stop=(cc == 1))
                ch = 4 * st + mc
                vpsv = vps[:, 0:256].rearrange("p (hp hl c) -> p hp hl c", hp=2, hl=2)
                nc.scalar.activation(out=v5[:, ch, :, :, 0:64], in_=vpsv, func=AF.Copy)

        def emit_q(pool):
            for w in range(3):
                qn = WINDOWS[w][1]
                for kb in range(2):
                    qps = pool.tile([128, 512], F32, tag="acc", name="qps")
                    for cc in range(2):
                        nc.tensor.matmul(
                            qps[:, 0:qn], lhsT=wT_h[:, cc, 128 * kb:128 * kb + 128],
                            rhs=xnq[:, cc, 512 * w:512 * w + qn], start=(cc == 0), stop=(cc == 1))
                    nc.scalar.activation(out=q_res[:, kb, w, 0:qn], in_=qps[:, 0:qn],
                                         func=AF.Identity, bias=pvec_sb[:, kb:kb + 1])

        # ---- attention ----
        o_tiles = {}

        def gen_attention(hp, w):
            """Generator: one yield per S-tile group (for interleaved emission).
            The window's epilogue is emitted inline right after the last group
            (o_t banks are double-buffered, so the next window never waits)."""
            q0, qn, kc0, nch = WINDOWS[w]
            o_t = {hl: ops.tile([128, 512], F32, tag=f"o{hl}", name=f"o{hl}") for hl in range(2)}
            o_tiles[(hp, w)] = o_t
            stream = [(hl, kc0 + c) for c in range(nch) for hl in range(2)]
            groups = [stream[i:i + SCHUNK] for i in range(0, len(stream), SCHUNK)]
            s_tiles = {}

            def emit_qk(g):
                # each QK matmul output must start on a PSUM bank boundary
                s_ps = sps.tile([128, 2, 512], F32, tag="s", name="s_ps")
                for j, (hl, kc) in enumerate(groups[g]):
                    nc.tensor.matmul(
                        s_ps[:, j, 0:qn],
                        lhsT=k_res[64 * hl:64 * hl + 64, hp, 128 * kc:128 * kc + 128],
                        rhs=q_res[64 * hl:64 * hl + 64, hp, w, 0:qn],
                        start=True, stop=True)
                s_tiles[g] = s_ps

            emit_qk(0)
            for g in range(len(groups)):
                if g + 1 < len(groups):
                    emit_qk(g + 1)
                items = groups[g]
                m = len(items)
                s_ps = s_tiles.pop(g)
                pt = ptp.tile([128, 2 * 512], F16, tag="p", name="pt")
                ptv = pt[:, 0:m * qn].rearrange("p (j c) -> p j c", j=m)
                nc.scalar.activation(out=ptv, in_=s_ps[:, 0:m, 0:qn], func=AF.Exp, scale=SCALE)
                for j, (hl, kc) in enumerate(items):
                    nc.tensor.matmul(
                        o_t[hl][0:65, 0:qn],
                        lhsT=v5[:, kc, hp, hl, :],
                        rhs=pt[:, qn * j:qn * (j + 1)],
                        start=(kc == kc0), stop=(kc == kc0 + nch - 1))
                yield
            make_epilogue(hp, w)()

        def drive(gen, inject=None):
            """Consume gen; inject[-1] fns emit before it starts, inject[g]
            right after group g (epilogues/projections of earlier windows, so
            they overlap this window's attention)."""
            inject = inject or {}
            for fn in inject.get(-1, []):
                fn()
            g = 0
            for _ in gen:
                for fn in inject.get(g, []):
                    fn()
                g += 1

        def make_epilogue(hp, w):
            """O^T[0:64] / O^T[64] -> attn[0:64, head, q]  (all on partitions 0:64)."""
            q0, qn, _, _ = WINDOWS[w]
            o_t = o_tiles[(hp, w)]

            def fn():
                for hl in range(2):
                    osb = epi.tile([65, 512], F32, tag="osb", name="osb")
                    nc.vector.tensor_copy(out=osb[:, 0:qn], in_=o_t[hl][0:65, 0:qn])
                    # sums row (f16, base partition 0) -> broadcast [64, qn]
                    # on PE (rank-1 with ones), then reciprocal+mult on DVE
                    # across all 64 lanes at once
                    srow = epi.tile([1, 512], F16, tag="srow", name="srow")
                    nc.vector.tensor_copy(out=srow[:, 0:qn], in_=osb[64:65, 0:qn])
                    # broadcast back into this o_t bank (osb copy freed it)
                    nc.tensor.matmul(o_t[hl][0:64, 0:qn], lhsT=ones_h[0:1, 0:64],
                                     rhs=srow[:, 0:qn], start=True, stop=True)
                    recb = epi.tile([64, 512], F32, tag="recb", name="recb")
                    nc.vector.reciprocal(out=recb[:, 0:qn], in_=o_t[hl][0:64, 0:qn])
                    nc.vector.tensor_tensor(
                        out=attn_sb[:, 2 * hp + hl, q0:q0 + qn],
                        in0=osb[0:64, 0:qn], in1=recb[:, 0:qn], op=ALU.mult)
            return fn

        def make_proj(w):
            q0, qn, _, _ = WINDOWS[w]

            def fn():
                for mc in range(2):
                    pp = o_tiles[(1, w)][mc]  # window w's freed o_t banks
                    nc.tensor.matmul(pp[:, 0:qn], lhsT=projbr_h[0:1, 128 * mc:128 * (mc + 1)],
                                     rhs=ones_h[0:1, 0:qn], start=True, stop=False)
                    for h in range(HEADS):
                        nc.tensor.matmul(pp[:, 0:qn], lhsT=projTh_h[:, h, 128 * mc:128 * (mc + 1)],
                                         rhs=attn_sb[:, h, q0:q0 + qn],
                                         start=False, stop=(h == HEADS - 1))
                    ot = outp.tile([128, 512], F32, tag="ot")
                    nc.vector.tensor_tensor(out=ot[:, 0:qn], in0=pp[:, 0:qn],
                                            in1=xq_sb[:, mc, q0:q0 + qn], op=ALU.add)
                    nc.sync.dma_start(out=out_d[128 * mc:128 * (mc + 1), q0:q0 + qn], in_=ot[:, 0:qn])
            return fn

        if ABLATE == "noqkv":
            return
        # qkv standalone with a deep (6-bank) PSUM rotation (PE never waits
        # on the ScalarE/DVE PSUM drains)
        with tc.tile_pool(name="qacc", bufs=6, space="PSUM") as qacc:
            emit_q(qacc)
            for st in range(NST):
                emit_qkv_k(st, qacc)
                emit_qkv_v(st, qacc)
        if ABLATE == "noattn":
            return
        # attention sweeps: o_t banks double-buffered so window w+1 starts
        # immediately; epilogues run inline, projections (sweep B) are
        # injected into the next window and accumulate in freed o_t banks.
        with (
            tc.tile_pool(name="sps", bufs=2, space="PSUM") as sps,
            tc.tile_pool(name="ops", bufs=2, space="PSUM") as ops,
        ):
            drive(gen_attention(0, 0))
            drive(gen_attention(0, 1))
            drive(gen_attention(0, 2))
            drive(gen_attention(1, 0))
            drive(gen_attention(1, 1), {3: [make_proj(0)]})
            drive(gen_attention(1, 2), {3: [make_proj(1)]})
            make_proj(2)()


def make_inputs(x, norm_w, norm_b, qkv_w, qkv_b, proj_w, proj_b):
    """Host-side prep: full-input numpy -> per-core in_maps."""
    x2 = np.ascontiguousarray(np.asarray(x, np.float32).reshape(C, SEQ))
    x16 = x2.astype(np.float16)
    qkv_w = np.asarray(qkv_w, np.float32)
    qkv_b = np.asarray(qkv_b, np.float32)
    proj_w = np.asarray(proj_w, np.float32)
    proj_b = np.asarray(proj_b, np.float32)
    norm_w = np.asarray(norm_w, np.float32)
    norm_b = np.asarray(norm_b, np.float32)

    wT = np.ascontiguousarray(qkv_w.T)
    projT = np.ascontiguousarray(proj_w.T)  # [c_in, c_out]
    projTh = np.ascontiguousarray(
        projT.reshape(4, 64, C).transpose(1, 0, 2).reshape(64, 4 * C))
    # v-bias folds into the projection bias: proj(attn + bv) = proj(attn) + proj_w @ bv
    projbr = (proj_b + proj_w @ qkv_b[2 * C:3 * C]).reshape(1, C).astype(np.float32)
    pvec = np.stack([
        qkv_b[0:128], qkv_b[128:256],            # q bias bank 0/1
        qkv_b[C:C + 128], qkv_b[C + 128:2 * C],  # k bias bank 0/1
        norm_w[0:128], norm_w[128:256],
        norm_b[0:128], norm_b[128:256],
    ], axis=1).astype(np.float32)
    cidx = np.arange(128)
    gidx = np.arange(16)
    G = ((cidx[:, None] // 8) == gidx[None, :]).astype(np.float32) / 8.0
    GT = np.ascontiguousarray(G.T * 8.0)

    common = dict(x=x16, wT=wT, projTh=projTh, pvec=pvec, projbr=projbr, G=G, GT=GT)
    in_maps = []
    cols = []
    for i in range(NCORES):
        ci = np.concatenate([
            np.arange(512 * i, 512 * (i + 1)),
            np.arange(4096 + 512 * i, 4096 + 512 * (i + 1)),
            np.arange(8192 + 128 * i, 8192 + 128 * (i + 1)),
        ])
        cols.append(ci)
        m = dict(common)
        m["xq"] = np.ascontiguousarray(x2[:, ci])
        in_maps.append(m)
    return in_maps, cols


_NC_CACHE = {}


def kernel(x, norm_w, norm_b, qkv_w, qkv_b, proj_w, proj_b):
    from concourse.bass_utils import run_bass_kernel_spmd

    _patch_tile_drain()
    _patch_to_json_split_waits()
    in_maps, cols = make_inputs(x, norm_w, norm_b, qkv_w, qkv_b, proj_w, proj_b)
    if "nc" not in _NC_CACHE:
        _NC_CACHE["nc"] = build_nc()
    nc = _NC_CACHE["nc"]
    res = run_bass_kernel_spmd(nc, in_maps, core_ids=list(range(NCORES)))
    out = np.zeros((C, SEQ), np.float32)
    for i in range(NCORES):
        out[:, cols[i]] = res.results[i]["out"]
    return out.reshape(1, C, 96, 96)
